# revision 2
# baseline (speedup 1.0000x reference)
"""EvoBinarizedLayer as one fp8 matmul per population member.

Math: per population p, with xb = unpacked bits of x (LSB-first) and
w0/w1 the two unpacked weight bit-planes,

  count[p] = xb @ w0 + (1 - xb) @ w1
           = xb @ (w0 - w1) + colsum(w1)

so each core computes a single [512,2048] @ [2048,2048] matmul with
lhs entries in {0,1} and rhs entries in {-1,0,1} (both exact in fp8
e4m3, accumulated exactly in fp32 PSUM), plus a per-(p,o) bias added
on the host. Counts <= 2048 are exact in fp16, so the device emits
fp16 and the host upcasts to int32.

Sharding: population dim P=8, one member per NeuronCore (x replicated).

Device schedule (from baseline trace analysis): the warm MM stream is
already at the per-instruction roofline (216 ns per N=512 DoubleRow
matmul, LDWEIGHTS hidden), so the wins are at the edges:
  - input DMAs issued in strict need-order at k-pair (128 KiB)
    granularity for the first o-quarter so the first real matmul
    starts ~2 us sooner (the 16-engine DMA pool serializes chunk
    arrivals; smaller first pieces = earlier first dependency);
  - PE warmup matmuls start immediately after the entry barrier
    (vector memset is the first op) and bridge until data arrives,
    opening the HAM clock gate before the real stream;
  - each [128,512] PSUM drain is CAST and DMA'd out immediately (no
    [128,1024] pair staging), and the very last batch-tile of the
    last o-quarter runs as two N=256 half-groups in separate PSUM
    banks so the final dependency chain is MM -> 350ns CAST -> 64KiB
    DMA instead of MM -> 685ns CAST -> 256KiB DMA.
"""

import numpy as np
import ml_dtypes

POP, BATCH, IN_INTS, OUT_F = 8, 512, 32, 2048
K = IN_INTS * 64          # 2048 contraction (bit) dim
KT = K // 128             # 16 k-tiles of 128
N_CORES = 8
N_WARM = 18

_FP8 = ml_dtypes.float8_e4m3

_cached = {}


def _build_nc():
    import concourse.tile as tile
    from concourse import bacc, mybir

    dt = mybir.dt
    nc = bacc.Bacc(
        "TRN2", target_bir_lowering=False, debug=False, num_devices=N_CORES
    )
    xbt_d = nc.dram_tensor(
        "xbt", [4, 128, 4, BATCH], dt.float8e4, kind="ExternalInput"
    ).ap()
    wd_d = nc.dram_tensor(
        "wd", [4, 4, 128, 4, 512], dt.float8e4, kind="ExternalInput"
    ).ap()
    out_d = nc.dram_tensor(
        "out", [BATCH, OUT_F], dt.float16, kind="ExternalOutput"
    ).ap()

    with tile.TileContext(nc) as tc:
        with (
            tc.tile_pool(name="xbt", bufs=1) as xbt_pool,
            tc.tile_pool(name="wd", bufs=1) as wd_pool,
            tc.tile_pool(name="outp", bufs=6) as out_pool,
            tc.tile_pool(name="psum", bufs=8, space="PSUM") as psum_pool,
        ):
            S, A, G = nc.sync, nc.scalar, nc.gpsimd
            DR = mybir.MatmulPerfMode.DoubleRow

            # xbt_sb[p, k, b]: bit row k*128+p, batch b
            xbt_sb = xbt_pool.tile([128, KT, BATCH], dt.float8e4)
            # wd_sb[p, ob, k, o']: bit row k*128+p, out feature ob*512+o'
            wd_sb = wd_pool.tile([128, 4, KT, 512], dt.float8e4)

            # PE warmup: dummy DoubleRow matmuls on a small zeroed tile so
            # the HAM clock-gate opens (K=8/8) before the real stream.
            # memset on vector = its first op, so warmups start right
            # after the entry barrier and bridge until the first input
            # pieces land.
            warm = xbt_pool.tile([128, 2, 128], dt.float8e4, tag="warm")
            nc.vector.memset(warm[:], 0.0)
            wps = psum_pool.tile([128, 512], dt.float32, tag="ps")
            for _ in range(N_WARM):
                nc.tensor.matmul(
                    wps[:, :128],
                    warm[:],
                    warm[:],
                    start=True,
                    stop=True,
                    perf_mode=DR,
                )

            # Input DMAs in strict need-order. The ob=0 pass consumes
            # (x k-pair, wd k-pair) in k order, so those stream first at
            # 128 KiB granularity (1 KiB per partition per piece); later
            # o-quarters use whole k-quarter chunks (256 KiB).
            def dma_x(e, kt0, kt1):
                if kt0 % 4 == 0 and kt1 % 4 == 0:
                    src = xbt_d[kt0 // 4 : kt1 // 4]
                else:
                    kq = kt0 // 4
                    src = xbt_d[kq][:, kt0 - 4 * kq : kt1 - 4 * kq, :]
                e.dma_start(xbt_sb[:, kt0:kt1, :], src)

            def dma_w(e, ob, kt0, kt1):
                kq = kt0 // 4
                if kt0 % 4 == 0 and kt1 == kt0 + 4:
                    src = wd_d[kq, ob]
                else:
                    src = wd_d[kq, ob][:, kt0 - 4 * kq : kt1 - 4 * kq, :]
                e.dma_start(wd_sb[:, ob, kt0:kt1, :], src)

            dma_w(S, 0, 0, 2)
            dma_x(A, 0, 2)
            dma_w(G, 0, 2, 4)
            dma_x(S, 2, 4)
            dma_w(A, 0, 4, 6)
            dma_x(G, 4, 6)
            dma_w(S, 0, 6, 8)
            dma_x(A, 6, 8)
            dma_w(G, 0, 8, 12)
            dma_x(S, 8, 12)
            dma_w(A, 0, 12, 16)
            dma_x(G, 12, 16)
            rr = [0]
            engines = [S, A, G]

            def next_engine():
                e = engines[rr[0] % 3]
                rr[0] += 1
                return e

            for ob in range(1, 4):
                for kq in range(4):
                    dma_w(next_engine(), ob, 4 * kq, 4 * (kq + 1))

            # Drain: CAST the [128,512] psum to fp16 and DMA it out
            # immediately (1 KiB runs per partition).
            def drain(ps, ob, bt, e):
                ot = out_pool.tile(
                    [128, 512], dt.float16, tag="ot", name=f"ot_{ob}_{bt}"
                )
                nc.vector.tensor_copy(ot[:], ps[:])
                e.dma_start(
                    out_d[
                        128 * bt : 128 * (bt + 1),
                        512 * ob : 512 * (ob + 1),
                    ],
                    ot[:],
                )

            # ob 0-2: k-outer across 4 concurrent psum banks (one per
            # batch tile) so each arriving chunk feeds 4 matmuls before
            # the next is needed (compute ramp matches DMA supply).
            for ob in range(3):
                pss = [
                    psum_pool.tile(
                        [128, 512], dt.float32, tag="ps", name=f"ps_{ob}_{bt}"
                    )
                    for bt in range(4)
                ]
                for k in range(KT // 2):
                    for bt in range(4):
                        nc.tensor.matmul(
                            pss[bt][:],
                            xbt_sb[:, 2 * k : 2 * k + 2, 128 * bt : 128 * (bt + 1)],
                            wd_sb[:, ob, 2 * k : 2 * k + 2, :],
                            start=(k == 0),
                            stop=(k == KT // 2 - 1),
                            perf_mode=DR,
                        )
                for bt in range(4):
                    drain(pss[bt], ob, bt, next_engine())

            # Final o-quarter: batch-tile-serial (k inner) so drains
            # stagger; the last batch tile runs as two N=256 half-groups
            # in separate PSUM banks for a short final drain chain.
            for bt in range(3):
                ps = psum_pool.tile(
                    [128, 512], dt.float32, tag="ps", name=f"ps_l{bt}"
                )
                for k in range(KT // 2):
                    nc.tensor.matmul(
                        ps[:],
                        xbt_sb[:, 2 * k : 2 * k + 2, 128 * bt : 128 * (bt + 1)],
                        wd_sb[:, 3, 2 * k : 2 * k + 2, :],
                        start=(k == 0),
                        stop=(k == KT // 2 - 1),
                        perf_mode=DR,
                    )
                drain(ps, 3, bt, next_engine())
            for half, e in ((0, A), (1, G)):
                ps = psum_pool.tile(
                    [128, 512], dt.float32, tag="ps", name=f"ps_h{half}"
                )
                for k in range(KT // 2):
                    nc.tensor.matmul(
                        ps[:, :256],
                        xbt_sb[:, 2 * k : 2 * k + 2, 384:512],
                        wd_sb[:, 3, 2 * k : 2 * k + 2, 256 * half : 256 * (half + 1)],
                        start=(k == 0),
                        stop=(k == KT // 2 - 1),
                        perf_mode=DR,
                    )
                ot = out_pool.tile(
                    [128, 256], dt.float16, tag="ot", name=f"ot_h{half}"
                )
                nc.vector.tensor_copy(ot[:], ps[:, :256])
                e.dma_start(
                    out_d[384:512, 1536 + 256 * half : 1536 + 256 * (half + 1)],
                    ot[:],
                )
    nc.compile()
    return nc


def get_nc():
    if "nc" not in _cached:
        _cached["nc"] = _build_nc()
    return _cached["nc"]


def pack_inputs(x, w):
    """Host-side bit unpack + layout. Returns (xbt, wd_cores, bias).

    xbt: [4, 128, 4, BATCH] fp8; xbt[kq, p, k', b] = bit (4kq+k')*128+p of x[b]
    wd_cores[p]: [4, 4, 128, 4, 512] fp8; [kq, ob, p, k', o'] =
        (w0-w1) at bit row (4kq+k')*128+p, out feature ob*512+o'
    bias: [POP, OUT_F] int32 colsum of w1 bits
    """
    xb = np.unpackbits(
        x.view(np.uint8).reshape(BATCH, IN_INTS, 8), axis=-1, bitorder="little"
    ).reshape(BATCH, K)
    xbt = np.ascontiguousarray(
        xb.T.reshape(4, 4, 128, BATCH).transpose(0, 2, 1, 3)
    ).astype(_FP8)

    wbits = np.unpackbits(
        w.view(np.uint8).reshape(POP, IN_INTS, 2, OUT_F, 8),
        axis=-1,
        bitorder="little",
    )  # [POP, IN_INTS, 2, OUT_F, 64]
    w0 = wbits[:, :, 0].transpose(0, 1, 3, 2).reshape(POP, K, OUT_F)
    w1 = wbits[:, :, 1].transpose(0, 1, 3, 2).reshape(POP, K, OUT_F)
    bias = w1.sum(axis=1, dtype=np.int32)  # [POP, OUT_F]
    wd = w0.astype(np.int8) - w1.astype(np.int8)  # {-1,0,1}
    wd_cores = [
        np.ascontiguousarray(
            wd[p].reshape(4, 4, 128, 4, 512).transpose(0, 3, 2, 1, 4)
        ).astype(_FP8)
        for p in range(POP)
    ]
    return xbt, wd_cores, bias


def kernel(x, w):
    from concourse.bass_utils import run_bass_kernel_spmd

    nc = get_nc()
    xbt, wd_cores, bias = pack_inputs(np.asarray(x), np.asarray(w))
    in_maps = [{"xbt": xbt, "wd": wd_cores[p]} for p in range(N_CORES)]
    try:
        res = run_bass_kernel_spmd(nc, in_maps, list(range(N_CORES)))
    except Exception:
        # NRT_EXEC_UNIT_UNRECOVERABLE has been observed transiently on this
        # fabric; one retry has always succeeded.
        res = run_bass_kernel_spmd(nc, in_maps, list(range(N_CORES)))
    out = np.empty((POP, BATCH, OUT_F), dtype=np.int32)
    for p in range(N_CORES):
        out[p] = res.results[p]["out"].astype(np.int32) + bias[p][None, :]
    return out


# revision 5
# speedup vs baseline: 1.1077x; 1.1077x over previous
"""EvoBinarizedLayer as one fp8 matmul per population member.

Math: per population p, with xb = unpacked bits of x (LSB-first) and
w0/w1 the two unpacked weight bit-planes,

  count[p] = xb @ w0 + (1 - xb) @ w1
           = xb @ (w0 - w1) + colsum(w1)

so each core computes a single [512,2048] @ [2048,2048] matmul with
lhs entries in {0,1} and rhs entries in {-1,0,1} (both exact in fp8
e4m3, accumulated exactly in fp32 PSUM), plus a per-(p,o) bias added
on the host. Counts <= 2048 are exact in fp16, so the device emits
fp16 and the host upcasts to int32.

Sharding: population dim P=8, one member per NeuronCore (x replicated).

Device schedule (from baseline trace analysis): the warm MM stream is
already at the per-instruction roofline (216 ns per N=512 DoubleRow
matmul, LDWEIGHTS hidden), so the wins are at the edges:
  - input DMAs issued in strict need-order at k-pair (128 KiB)
    granularity for the first o-quarter so the first real matmul
    starts ~2 us sooner (the 16-engine DMA pool serializes chunk
    arrivals; smaller first pieces = earlier first dependency);
  - PE warmup matmuls start immediately after the entry barrier
    (vector memset is the first op) and bridge until data arrives,
    opening the HAM clock gate before the real stream;
  - each [128,512] PSUM drain is CAST and DMA'd out immediately (no
    [128,1024] pair staging), and the very last batch-tile of the
    last o-quarter runs as two N=256 half-groups in separate PSUM
    banks so the final dependency chain is MM -> 350ns CAST -> 64KiB
    DMA instead of MM -> 685ns CAST -> 256KiB DMA.
"""

import numpy as np
import ml_dtypes

POP, BATCH, IN_INTS, OUT_F = 8, 512, 32, 2048
K = IN_INTS * 64          # 2048 contraction (bit) dim
KT = K // 128             # 16 k-tiles of 128
N_CORES = 8
N_WARM = 18

_FP8 = ml_dtypes.float8_e4m3

_cached = {}


def _build_nc():
    import concourse.tile as tile
    from concourse import bacc, mybir

    dt = mybir.dt
    nc = bacc.Bacc(
        "TRN2", target_bir_lowering=False, debug=False, num_devices=N_CORES
    )
    xbt_d = nc.dram_tensor(
        "xbt", [4, 128, 4, BATCH], dt.float8e4, kind="ExternalInput"
    ).ap()
    wd_d = nc.dram_tensor(
        "wd", [4, 4, 128, 4, 512], dt.float8e4, kind="ExternalInput"
    ).ap()
    out_d = nc.dram_tensor(
        "out", [BATCH, OUT_F], dt.float16, kind="ExternalOutput"
    ).ap()

    with tile.TileContext(nc) as tc:
        with (
            tc.tile_pool(name="xbt", bufs=1) as xbt_pool,
            tc.tile_pool(name="wd", bufs=1) as wd_pool,
            tc.tile_pool(name="outp", bufs=6) as out_pool,
            tc.tile_pool(name="psum", bufs=8, space="PSUM") as psum_pool,
        ):
            S, A, G = nc.sync, nc.scalar, nc.gpsimd
            DR = mybir.MatmulPerfMode.DoubleRow

            # xbt_sb[p, k, b]: bit row k*128+p, batch b
            xbt_sb = xbt_pool.tile([128, KT, BATCH], dt.float8e4)
            # wd_sb[p, ob, k, o']: bit row k*128+p, out feature ob*512+o'
            wd_sb = wd_pool.tile([128, 4, KT, 512], dt.float8e4)

            # PE warmup: dummy DoubleRow matmuls on a small zeroed tile so
            # the HAM clock-gate opens (K=8/8) before the real stream.
            # memset on vector = its first op, so warmups start right
            # after the entry barrier and bridge until the first input
            # pieces land.
            warm = xbt_pool.tile([128, 2, 128], dt.float8e4, tag="warm")
            nc.vector.memset(warm[:], 0.0)
            wps = psum_pool.tile([128, 512], dt.float32, tag="ps")
            for _ in range(N_WARM):
                nc.tensor.matmul(
                    wps[:, :128],
                    warm[:],
                    warm[:],
                    start=True,
                    stop=True,
                    perf_mode=DR,
                )

            # Input DMAs in strict need-order. The ob=0 pass consumes
            # (x k-pair, wd k-pair) in k order, so those stream first at
            # 128 KiB granularity (1 KiB per partition per piece); later
            # o-quarters use whole k-quarter chunks (256 KiB).
            def dma_x(e, kt0, kt1):
                if kt0 % 4 == 0 and kt1 % 4 == 0:
                    src = xbt_d[kt0 // 4 : kt1 // 4]
                else:
                    kq = kt0 // 4
                    src = xbt_d[kq][:, kt0 - 4 * kq : kt1 - 4 * kq, :]
                e.dma_start(xbt_sb[:, kt0:kt1, :], src)

            def dma_w(e, ob, kt0, kt1):
                kq = kt0 // 4
                if kt0 % 4 == 0 and kt1 == kt0 + 4:
                    src = wd_d[kq, ob]
                else:
                    src = wd_d[kq, ob][:, kt0 - 4 * kq : kt1 - 4 * kq, :]
                e.dma_start(wd_sb[:, ob, kt0:kt1, :], src)

            dma_w(S, 0, 0, 2)
            dma_x(A, 0, 2)
            dma_w(G, 0, 2, 4)
            dma_x(S, 2, 4)
            dma_w(A, 0, 4, 8)
            dma_x(G, 4, 8)
            dma_w(S, 0, 8, 12)
            dma_x(A, 8, 12)
            dma_w(G, 0, 12, 16)
            dma_x(S, 12, 16)
            rr = [0]
            engines = [S, A, G]

            def next_engine():
                e = engines[rr[0] % 3]
                rr[0] += 1
                return e

            for ob in range(1, 4):
                for kq in range(4):
                    dma_w(next_engine(), ob, 4 * kq, 4 * (kq + 1))

            # Drain: CAST the [128,512] psum into a [128,1024] o-pair
            # staging tile; DMA the pair out (2 KiB runs per partition)
            # once its second half lands. Exception: (bt=3, ob=2) goes
            # out on its own right away so the final-tile staging only
            # carries the two small ob=3 halves.
            stage = {}

            def drain(ps, ob, bt, e):
                half = ob % 2
                if half == 0:
                    stage[bt] = out_pool.tile(
                        [128, 1024], dt.float16, tag="ot", name=f"ot_{ob}_{bt}"
                    )
                nc.vector.tensor_copy(
                    stage[bt][:, 512 * half : 512 * (half + 1)], ps[:]
                )
                if half == 1:
                    e.dma_start(
                        out_d[
                            128 * bt : 128 * (bt + 1),
                            1024 * (ob // 2) : 1024 * (ob // 2 + 1),
                        ],
                        stage[bt][:],
                    )

            def drain_single(ps, ob, bt, e):
                ot = out_pool.tile(
                    [128, 512], dt.float16, tag="ot", name=f"ot_{ob}_{bt}"
                )
                nc.vector.tensor_copy(ot[:], ps[:])
                e.dma_start(
                    out_d[
                        128 * bt : 128 * (bt + 1),
                        512 * ob : 512 * (ob + 1),
                    ],
                    ot[:],
                )

            # ob 0-2: k-outer across 4 concurrent psum banks (one per
            # batch tile) so each arriving chunk feeds 4 matmuls before
            # the next is needed (compute ramp matches DMA supply).
            for ob in range(3):
                pss = [
                    psum_pool.tile(
                        [128, 512], dt.float32, tag="ps", name=f"ps_{ob}_{bt}"
                    )
                    for bt in range(4)
                ]
                for k in range(KT // 2):
                    for bt in range(4):
                        nc.tensor.matmul(
                            pss[bt][:],
                            xbt_sb[:, 2 * k : 2 * k + 2, 128 * bt : 128 * (bt + 1)],
                            wd_sb[:, ob, 2 * k : 2 * k + 2, :],
                            start=(k == 0),
                            stop=(k == KT // 2 - 1),
                            perf_mode=DR,
                        )
                for bt in range(4):
                    if ob == 2 and bt == 3:
                        drain_single(pss[bt], ob, bt, next_engine())
                    else:
                        drain(pss[bt], ob, bt, next_engine())

            # Final o-quarter: batch-tile-serial (k inner) so drains
            # stagger; the last batch tile runs as two N=256 half-groups
            # in separate PSUM banks for a short final drain chain.
            for bt in range(3):
                ps = psum_pool.tile(
                    [128, 512], dt.float32, tag="ps", name=f"ps_l{bt}"
                )
                for k in range(KT // 2):
                    nc.tensor.matmul(
                        ps[:],
                        xbt_sb[:, 2 * k : 2 * k + 2, 128 * bt : 128 * (bt + 1)],
                        wd_sb[:, 3, 2 * k : 2 * k + 2, :],
                        start=(k == 0),
                        stop=(k == KT // 2 - 1),
                        perf_mode=DR,
                    )
                drain(ps, 3, bt, next_engine())
            for half, e in ((0, A), (1, G)):
                ps = psum_pool.tile(
                    [128, 512], dt.float32, tag="ps", name=f"ps_h{half}"
                )
                for k in range(KT // 2):
                    nc.tensor.matmul(
                        ps[:, :256],
                        xbt_sb[:, 2 * k : 2 * k + 2, 384:512],
                        wd_sb[:, 3, 2 * k : 2 * k + 2, 256 * half : 256 * (half + 1)],
                        start=(k == 0),
                        stop=(k == KT // 2 - 1),
                        perf_mode=DR,
                    )
                ot = out_pool.tile(
                    [128, 256], dt.float16, tag="ot", name=f"ot_h{half}"
                )
                nc.vector.tensor_copy(ot[:], ps[:, :256])
                e.dma_start(
                    out_d[384:512, 1536 + 256 * half : 1536 + 256 * (half + 1)],
                    ot[:],
                )
    nc.compile()
    return nc


def get_nc():
    if "nc" not in _cached:
        _cached["nc"] = _build_nc()
    return _cached["nc"]


def pack_inputs(x, w):
    """Host-side bit unpack + layout. Returns (xbt, wd_cores, bias).

    xbt: [4, 128, 4, BATCH] fp8; xbt[kq, p, k', b] = bit (4kq+k')*128+p of x[b]
    wd_cores[p]: [4, 4, 128, 4, 512] fp8; [kq, ob, p, k', o'] =
        (w0-w1) at bit row (4kq+k')*128+p, out feature ob*512+o'
    bias: [POP, OUT_F] int32 colsum of w1 bits
    """
    xb = np.unpackbits(
        x.view(np.uint8).reshape(BATCH, IN_INTS, 8), axis=-1, bitorder="little"
    ).reshape(BATCH, K)
    xbt = np.ascontiguousarray(
        xb.T.reshape(4, 4, 128, BATCH).transpose(0, 2, 1, 3)
    ).astype(_FP8)

    wbits = np.unpackbits(
        w.view(np.uint8).reshape(POP, IN_INTS, 2, OUT_F, 8),
        axis=-1,
        bitorder="little",
    )  # [POP, IN_INTS, 2, OUT_F, 64]
    w0 = wbits[:, :, 0].transpose(0, 1, 3, 2).reshape(POP, K, OUT_F)
    w1 = wbits[:, :, 1].transpose(0, 1, 3, 2).reshape(POP, K, OUT_F)
    bias = w1.sum(axis=1, dtype=np.int32)  # [POP, OUT_F]
    wd = w0.astype(np.int8) - w1.astype(np.int8)  # {-1,0,1}
    wd_cores = [
        np.ascontiguousarray(
            wd[p].reshape(4, 4, 128, 4, 512).transpose(0, 3, 2, 1, 4)
        ).astype(_FP8)
        for p in range(POP)
    ]
    return xbt, wd_cores, bias


def kernel(x, w):
    from concourse.bass_utils import run_bass_kernel_spmd

    nc = get_nc()
    xbt, wd_cores, bias = pack_inputs(np.asarray(x), np.asarray(w))
    in_maps = [{"xbt": xbt, "wd": wd_cores[p]} for p in range(N_CORES)]
    try:
        res = run_bass_kernel_spmd(nc, in_maps, list(range(N_CORES)))
    except Exception:
        # NRT_EXEC_UNIT_UNRECOVERABLE has been observed transiently on this
        # fabric; one retry has always succeeded.
        res = run_bass_kernel_spmd(nc, in_maps, list(range(N_CORES)))
    out = np.empty((POP, BATCH, OUT_F), dtype=np.int32)
    for p in range(N_CORES):
        out[p] = res.results[p]["out"].astype(np.int32) + bias[p][None, :]
    return out


# revision 8
# speedup vs baseline: 1.1163x; 1.0078x over previous
"""EvoBinarizedLayer as one fp8 matmul per population member.

Math: per population p, with xb = unpacked bits of x (LSB-first) and
w0/w1 the two unpacked weight bit-planes,

  count[p] = xb @ w0 + (1 - xb) @ w1
           = xb @ (w0 - w1) + colsum(w1)

so each core computes a single [512,2048] @ [2048,2048] matmul with
lhs entries in {0,1} and rhs entries in {-1,0,1} (both exact in fp8
e4m3, accumulated exactly in fp32 PSUM), plus a per-(p,o) bias added
on the host. Counts <= 2048 are exact in fp16, so the device emits
fp16 and the host upcasts to int32.

Sharding: population dim P=8, one member per NeuronCore (x replicated).

Device schedule (from baseline trace analysis): the warm MM stream is
already at the per-instruction roofline (216 ns per N=512 DoubleRow
matmul, LDWEIGHTS hidden), so the wins are at the edges:
  - input DMAs issued in strict need-order at k-pair (128 KiB)
    granularity for the first o-quarter so the first real matmul
    starts ~2 us sooner (the 16-engine DMA pool serializes chunk
    arrivals; smaller first pieces = earlier first dependency);
  - PE warmup matmuls start immediately after the entry barrier
    (vector memset is the first op) and bridge until data arrives,
    opening the HAM clock gate before the real stream;
  - each [128,512] PSUM drain is CAST and DMA'd out immediately (no
    [128,1024] pair staging), and the very last batch-tile of the
    last o-quarter runs as two N=256 half-groups in separate PSUM
    banks so the final dependency chain is MM -> 350ns CAST -> 64KiB
    DMA instead of MM -> 685ns CAST -> 256KiB DMA.
"""

import numpy as np
import ml_dtypes

POP, BATCH, IN_INTS, OUT_F = 8, 512, 32, 2048
K = IN_INTS * 64          # 2048 contraction (bit) dim
KT = K // 128             # 16 k-tiles of 128
N_CORES = 8
N_WARM = 18

_FP8 = ml_dtypes.float8_e4m3

_cached = {}


def _build_nc():
    import concourse.tile as tile
    from concourse import bacc, mybir

    dt = mybir.dt
    nc = bacc.Bacc(
        "TRN2", target_bir_lowering=False, debug=False, num_devices=N_CORES
    )
    xbt_d = nc.dram_tensor(
        "xbt", [4, 128, 4, BATCH], dt.float8e4, kind="ExternalInput"
    ).ap()
    wd_d = nc.dram_tensor(
        "wd", [4, 4, 128, 4, 512], dt.float8e4, kind="ExternalInput"
    ).ap()
    out_d = nc.dram_tensor(
        "out", [BATCH, OUT_F], dt.float16, kind="ExternalOutput"
    ).ap()

    with tile.TileContext(nc) as tc:
        with (
            tc.tile_pool(name="xbt", bufs=1) as xbt_pool,
            tc.tile_pool(name="wd", bufs=1) as wd_pool,
            tc.tile_pool(name="outp", bufs=6) as out_pool,
            tc.tile_pool(name="psum", bufs=8, space="PSUM") as psum_pool,
        ):
            S, A, G = nc.sync, nc.scalar, nc.gpsimd
            DR = mybir.MatmulPerfMode.DoubleRow

            # xbt_sb[p, k, b]: bit row k*128+p, batch b
            xbt_sb = xbt_pool.tile([128, KT, BATCH], dt.float8e4)
            # wd_sb[p, ob, k, o']: bit row k*128+p, out feature ob*512+o'
            wd_sb = wd_pool.tile([128, 4, KT, 512], dt.float8e4)

            # PE warmup: dummy DoubleRow matmuls on a small zeroed tile so
            # the HAM clock-gate opens (K=8/8) before the real stream.
            # memset on vector = its first op, so warmups start right
            # after the entry barrier and bridge until the first input
            # pieces land.
            warm = xbt_pool.tile([128, 2, 128], dt.float8e4, tag="warm")
            nc.vector.memset(warm[:], 0.0)
            wps = psum_pool.tile([128, 512], dt.float32, tag="ps")
            for _ in range(N_WARM):
                nc.tensor.matmul(
                    wps[:, :128],
                    warm[:],
                    warm[:],
                    start=True,
                    stop=True,
                    perf_mode=DR,
                )

            # Input DMAs in strict need-order. The ob=0 pass consumes
            # (x k-pair, wd k-pair) in k order, so those stream first at
            # 128 KiB granularity (1 KiB per partition per piece); later
            # o-quarters use whole k-quarter chunks (256 KiB).
            def dma_x(e, kt0, kt1):
                if kt0 % 4 == 0 and kt1 % 4 == 0:
                    src = xbt_d[kt0 // 4 : kt1 // 4]
                else:
                    kq = kt0 // 4
                    src = xbt_d[kq][:, kt0 - 4 * kq : kt1 - 4 * kq, :]
                e.dma_start(xbt_sb[:, kt0:kt1, :], src)

            def dma_w(e, ob, kt0, kt1):
                kq = kt0 // 4
                if kt0 % 4 == 0 and kt1 == kt0 + 4:
                    src = wd_d[kq, ob]
                else:
                    src = wd_d[kq, ob][:, kt0 - 4 * kq : kt1 - 4 * kq, :]
                e.dma_start(wd_sb[:, ob, kt0:kt1, :], src)

            rr = [0]
            engines = [S, A, G]

            def next_engine():
                e = engines[rr[0] % 3]
                rr[0] += 1
                return e

            # ob=0 and x stream at k-pair granularity (128 KiB pieces,
            # x before w at each k since LDWEIGHTS pulls ahead),
            # round-robin across all three queues so each piece lands
            # ~1.2-1.4 us apart per queue — ahead of the compute ramp's
            # per-k-pair demand. Later o-quarters use 256 KiB chunks.
            for j in range(8):
                dma_x(next_engine(), 2 * j, 2 * j + 2)
                dma_w(next_engine(), 0, 2 * j, 2 * j + 2)
            for ob in range(1, 4):
                for kq in range(4):
                    dma_w(next_engine(), ob, 4 * kq, 4 * (kq + 1))

            # Drain: CAST the [128,512] psum into a [128,1024] o-pair
            # staging tile; DMA the pair out (2 KiB runs per partition)
            # once its second half lands. Exception: (bt=3, ob=2) goes
            # out on its own right away so the final-tile staging only
            # carries the two small ob=3 halves.
            stage = {}

            def drain(ps, ob, bt, e):
                half = ob % 2
                if half == 0:
                    stage[bt] = out_pool.tile(
                        [128, 1024], dt.float16, tag="ot", name=f"ot_{ob}_{bt}"
                    )
                nc.vector.tensor_copy(
                    stage[bt][:, 512 * half : 512 * (half + 1)], ps[:]
                )
                if half == 1:
                    e.dma_start(
                        out_d[
                            128 * bt : 128 * (bt + 1),
                            1024 * (ob // 2) : 1024 * (ob // 2 + 1),
                        ],
                        stage[bt][:],
                    )

            def drain_single(ps, ob, bt, e):
                ot = out_pool.tile(
                    [128, 512], dt.float16, tag="ot", name=f"ot_{ob}_{bt}"
                )
                nc.vector.tensor_copy(ot[:], ps[:])
                e.dma_start(
                    out_d[
                        128 * bt : 128 * (bt + 1),
                        512 * ob : 512 * (ob + 1),
                    ],
                    ot[:],
                )

            # ob 0-2: k-outer across 4 concurrent psum banks (one per
            # batch tile) so each arriving chunk feeds 4 matmuls before
            # the next is needed (compute ramp matches DMA supply).
            for ob in range(3):
                pss = [
                    psum_pool.tile(
                        [128, 512], dt.float32, tag="ps", name=f"ps_{ob}_{bt}"
                    )
                    for bt in range(4)
                ]
                for k in range(KT // 2):
                    for bt in range(4):
                        nc.tensor.matmul(
                            pss[bt][:],
                            xbt_sb[:, 2 * k : 2 * k + 2, 128 * bt : 128 * (bt + 1)],
                            wd_sb[:, ob, 2 * k : 2 * k + 2, :],
                            start=(k == 0),
                            stop=(k == KT // 2 - 1),
                            perf_mode=DR,
                        )
                for bt in range(4):
                    if ob == 2:
                        # ob2 goes out immediately as singles so the
                        # final o-quarter's drains are small and the
                        # HWDGE queues are clear at the end.
                        drain_single(pss[bt], ob, bt, (S, A)[bt % 2])
                    else:
                        drain(pss[bt], ob, bt, next_engine())

            # Final o-quarter: batch-tile-serial (k inner) so drains
            # stagger; the last batch tile runs as two N=256 half-groups
            # in separate PSUM banks for a short final drain chain.
            for bt in range(3):
                ps = psum_pool.tile(
                    [128, 512], dt.float32, tag="ps", name=f"ps_l{bt}"
                )
                for k in range(KT // 2):
                    nc.tensor.matmul(
                        ps[:],
                        xbt_sb[:, 2 * k : 2 * k + 2, 128 * bt : 128 * (bt + 1)],
                        wd_sb[:, 3, 2 * k : 2 * k + 2, :],
                        start=(k == 0),
                        stop=(k == KT // 2 - 1),
                        perf_mode=DR,
                    )
                drain_single(ps, 3, bt, (A, S)[bt % 2])
            for half, e in ((0, A), (1, S)):
                ps = psum_pool.tile(
                    [128, 512], dt.float32, tag="ps", name=f"ps_h{half}"
                )
                for k in range(KT // 2):
                    nc.tensor.matmul(
                        ps[:, :256],
                        xbt_sb[:, 2 * k : 2 * k + 2, 384:512],
                        wd_sb[:, 3, 2 * k : 2 * k + 2, 256 * half : 256 * (half + 1)],
                        start=(k == 0),
                        stop=(k == KT // 2 - 1),
                        perf_mode=DR,
                    )
                ot = out_pool.tile(
                    [128, 256], dt.float16, tag="ot", name=f"ot_h{half}"
                )
                nc.vector.tensor_copy(ot[:], ps[:, :256])
                e.dma_start(
                    out_d[384:512, 1536 + 256 * half : 1536 + 256 * (half + 1)],
                    ot[:],
                )
    nc.compile()
    return nc


def get_nc():
    if "nc" not in _cached:
        _cached["nc"] = _build_nc()
    return _cached["nc"]


def pack_inputs(x, w):
    """Host-side bit unpack + layout. Returns (xbt, wd_cores, bias).

    xbt: [4, 128, 4, BATCH] fp8; xbt[kq, p, k', b] = bit (4kq+k')*128+p of x[b]
    wd_cores[p]: [4, 4, 128, 4, 512] fp8; [kq, ob, p, k', o'] =
        (w0-w1) at bit row (4kq+k')*128+p, out feature ob*512+o'
    bias: [POP, OUT_F] int32 colsum of w1 bits
    """
    xb = np.unpackbits(
        x.view(np.uint8).reshape(BATCH, IN_INTS, 8), axis=-1, bitorder="little"
    ).reshape(BATCH, K)
    xbt = np.ascontiguousarray(
        xb.T.reshape(4, 4, 128, BATCH).transpose(0, 2, 1, 3)
    ).astype(_FP8)

    wbits = np.unpackbits(
        w.view(np.uint8).reshape(POP, IN_INTS, 2, OUT_F, 8),
        axis=-1,
        bitorder="little",
    )  # [POP, IN_INTS, 2, OUT_F, 64]
    w0 = wbits[:, :, 0].transpose(0, 1, 3, 2).reshape(POP, K, OUT_F)
    w1 = wbits[:, :, 1].transpose(0, 1, 3, 2).reshape(POP, K, OUT_F)
    bias = w1.sum(axis=1, dtype=np.int32)  # [POP, OUT_F]
    wd = w0.astype(np.int8) - w1.astype(np.int8)  # {-1,0,1}
    wd_cores = [
        np.ascontiguousarray(
            wd[p].reshape(4, 4, 128, 4, 512).transpose(0, 3, 2, 1, 4)
        ).astype(_FP8)
        for p in range(POP)
    ]
    return xbt, wd_cores, bias


def kernel(x, w):
    from concourse.bass_utils import run_bass_kernel_spmd

    nc = get_nc()
    xbt, wd_cores, bias = pack_inputs(np.asarray(x), np.asarray(w))
    in_maps = [{"xbt": xbt, "wd": wd_cores[p]} for p in range(N_CORES)]
    try:
        res = run_bass_kernel_spmd(nc, in_maps, list(range(N_CORES)))
    except Exception:
        # NRT_EXEC_UNIT_UNRECOVERABLE has been observed transiently on this
        # fabric; one retry has always succeeded.
        res = run_bass_kernel_spmd(nc, in_maps, list(range(N_CORES)))
    out = np.empty((POP, BATCH, OUT_F), dtype=np.int32)
    for p in range(N_CORES):
        out[p] = res.results[p]["out"].astype(np.int32) + bias[p][None, :]
    return out


# revision 10
# speedup vs baseline: 1.1595x; 1.0386x over previous
"""EvoBinarizedLayer as one fp8 matmul per population member.

Math: per population p, with xb = unpacked bits of x (LSB-first) and
w0/w1 the two unpacked weight bit-planes,

  count[p] = xb @ w0 + (1 - xb) @ w1
           = xb @ (w0 - w1) + colsum(w1)

so each core computes a single [512,2048] @ [2048,2048] matmul with
lhs entries in {0,1} and rhs entries in {-1,0,1} (both exact in fp8
e4m3, accumulated exactly in fp32 PSUM), plus a per-(p,o) bias added
on the host. Counts <= 2048 are exact in fp16, so the device emits
fp16 and the host upcasts to int32.

Sharding: population dim P=8, one member per NeuronCore (x replicated).

Device schedule (from baseline trace analysis): the warm MM stream is
already at the per-instruction roofline (216 ns per N=512 DoubleRow
matmul, LDWEIGHTS hidden), so the wins are at the edges:
  - input DMAs issued in strict need-order on the two HWDGE queues
    with per-partition-contiguous DRAM layouts (2+ KiB packets keep
    each queue at ~165 KB/us; gpsimd software-DGE is unused since its
    ring drain costs a fixed ~2.2 us in the exit barrier);
  - PE warmup matmuls start immediately after the entry barrier
    (vector memset is the first op) and bridge until data arrives,
    opening the HAM clock gate before the real stream;
  - each [128,512] PSUM drain is CAST and DMA'd out immediately (no
    [128,1024] pair staging), and the very last batch-tile of the
    last o-quarter runs as two N=256 half-groups in separate PSUM
    banks so the final dependency chain is MM -> 350ns CAST -> 64KiB
    DMA instead of MM -> 685ns CAST -> 256KiB DMA.
"""

import numpy as np
import ml_dtypes

POP, BATCH, IN_INTS, OUT_F = 8, 512, 32, 2048
K = IN_INTS * 64          # 2048 contraction (bit) dim
KT = K // 128             # 16 k-tiles of 128
N_CORES = 8
N_WARM = 22

_FP8 = ml_dtypes.float8_e4m3

_cached = {}


def _build_nc():
    import concourse.tile as tile
    from concourse import bacc, mybir

    dt = mybir.dt
    nc = bacc.Bacc(
        "TRN2", target_bir_lowering=False, debug=False, num_devices=N_CORES
    )
    xbt_d = nc.dram_tensor(
        "xbt", [128, KT, BATCH], dt.float8e4, kind="ExternalInput"
    ).ap()
    wd_d = nc.dram_tensor(
        "wd", [4, 128, KT, 512], dt.float8e4, kind="ExternalInput"
    ).ap()
    out_d = nc.dram_tensor(
        "out", [BATCH, OUT_F], dt.float16, kind="ExternalOutput"
    ).ap()

    with tile.TileContext(nc) as tc:
        with (
            tc.tile_pool(name="xbt", bufs=1) as xbt_pool,
            tc.tile_pool(name="wd", bufs=1) as wd_pool,
            tc.tile_pool(name="outp", bufs=6) as out_pool,
            tc.tile_pool(name="psum", bufs=8, space="PSUM") as psum_pool,
        ):
            S, A, G = nc.sync, nc.scalar, nc.gpsimd
            DR = mybir.MatmulPerfMode.DoubleRow

            # xbt_sb[p, k, b]: bit row k*128+p, batch b
            xbt_sb = xbt_pool.tile([128, KT, BATCH], dt.float8e4)
            # wd_sb[p, ob, k, o']: bit row k*128+p, out feature ob*512+o'
            wd_sb = wd_pool.tile([128, 4, KT, 512], dt.float8e4)

            # PE warmup: dummy DoubleRow matmuls on a small zeroed tile so
            # the HAM clock-gate opens (K=8/8) before the real stream.
            # memset on vector = its first op, so warmups start right
            # after the entry barrier and bridge until the first input
            # pieces land.
            warm = xbt_pool.tile([128, 2, 128], dt.float8e4, tag="warm")
            nc.vector.memset(warm[:], 0.0)
            wps = psum_pool.tile([128, 512], dt.float32, tag="ps")
            for _ in range(N_WARM):
                nc.tensor.matmul(
                    wps[:, :128],
                    warm[:],
                    warm[:],
                    start=True,
                    stop=True,
                    perf_mode=DR,
                )

            def dma_x(e, kt0, kt1):
                e.dma_start(xbt_sb[:, kt0:kt1, :], xbt_d[:, kt0:kt1, :])

            def dma_w(e, ob, kt0, kt1):
                e.dma_start(wd_sb[:, ob, kt0:kt1, :], wd_d[ob][:, kt0:kt1, :])

            rr = [0]
            engines = [S, A]

            def next_engine():
                e = engines[rr[0] % 2]
                rr[0] += 1
                return e

            # All DMAs ride the two HWDGE queues (sync/scalar): gpsimd's
            # software-DGE ring costs a fixed ~2.2us drain in the exit
            # barrier, and with per-partition-contiguous DRAM layouts the
            # HWDGE queues sustain ~165 KB/us each (2 KiB packets) vs the
            # ~300 KB/us stream demand. Need-order at 256 KiB k-quarter
            # granularity, x/w alternating across the two queues.
            for kq in range(4):
                dma_x(next_engine(), 4 * kq, 4 * (kq + 1))
                dma_w(next_engine(), 0, 4 * kq, 4 * (kq + 1))
            for ob in range(1, 4):
                for kq in range(4):
                    dma_w(next_engine(), ob, 4 * kq, 4 * (kq + 1))

            # Drain: CAST the [128,512] psum into a [128,1024] o-pair
            # staging tile; DMA the pair out (2 KiB runs per partition)
            # once its second half lands. Exception: (bt=3, ob=2) goes
            # out on its own right away so the final-tile staging only
            # carries the two small ob=3 halves.
            stage = {}

            def drain(ps, ob, bt, e):
                half = ob % 2
                if half == 0:
                    stage[bt] = out_pool.tile(
                        [128, 1024], dt.float16, tag="ot", name=f"ot_{ob}_{bt}"
                    )
                nc.vector.tensor_copy(
                    stage[bt][:, 512 * half : 512 * (half + 1)], ps[:]
                )
                if half == 1:
                    e.dma_start(
                        out_d[
                            128 * bt : 128 * (bt + 1),
                            1024 * (ob // 2) : 1024 * (ob // 2 + 1),
                        ],
                        stage[bt][:],
                    )

            def drain_single(ps, ob, bt, e):
                ot = out_pool.tile(
                    [128, 512], dt.float16, tag="ot", name=f"ot_{ob}_{bt}"
                )
                nc.vector.tensor_copy(ot[:], ps[:])
                e.dma_start(
                    out_d[
                        128 * bt : 128 * (bt + 1),
                        512 * ob : 512 * (ob + 1),
                    ],
                    ot[:],
                )

            # ob 0-2: k-outer across 4 concurrent psum banks (one per
            # batch tile) so each arriving chunk feeds 4 matmuls before
            # the next is needed (compute ramp matches DMA supply).
            for ob in range(3):
                pss = [
                    psum_pool.tile(
                        [128, 512], dt.float32, tag="ps", name=f"ps_{ob}_{bt}"
                    )
                    for bt in range(4)
                ]
                for k in range(KT // 2):
                    for bt in range(4):
                        nc.tensor.matmul(
                            pss[bt][:],
                            xbt_sb[:, 2 * k : 2 * k + 2, 128 * bt : 128 * (bt + 1)],
                            wd_sb[:, ob, 2 * k : 2 * k + 2, :],
                            start=(k == 0),
                            stop=(k == KT // 2 - 1),
                            perf_mode=DR,
                        )
                for bt in range(4):
                    if ob == 2:
                        # ob2 goes out immediately as singles so the
                        # final o-quarter's drains are small and the
                        # HWDGE queues are clear at the end.
                        drain_single(pss[bt], ob, bt, (A, S)[bt % 2])
                    else:
                        drain(pss[bt], ob, bt, next_engine())

            # Final o-quarter: batch-tile-serial (k inner) so drains
            # stagger; the last batch tile runs as two N=256 half-groups
            # in separate PSUM banks for a short final drain chain.
            for bt in range(3):
                ps = psum_pool.tile(
                    [128, 512], dt.float32, tag="ps", name=f"ps_l{bt}"
                )
                for k in range(KT // 2):
                    nc.tensor.matmul(
                        ps[:],
                        xbt_sb[:, 2 * k : 2 * k + 2, 128 * bt : 128 * (bt + 1)],
                        wd_sb[:, 3, 2 * k : 2 * k + 2, :],
                        start=(k == 0),
                        stop=(k == KT // 2 - 1),
                        perf_mode=DR,
                    )
                drain_single(ps, 3, bt, (A, A, S)[bt])
            for half, e in ((0, S), (1, S)):
                ps = psum_pool.tile(
                    [128, 512], dt.float32, tag="ps", name=f"ps_h{half}"
                )
                for k in range(KT // 2):
                    nc.tensor.matmul(
                        ps[:, :256],
                        xbt_sb[:, 2 * k : 2 * k + 2, 384:512],
                        wd_sb[:, 3, 2 * k : 2 * k + 2, 256 * half : 256 * (half + 1)],
                        start=(k == 0),
                        stop=(k == KT // 2 - 1),
                        perf_mode=DR,
                    )
                ot = out_pool.tile(
                    [128, 256], dt.float16, tag="ot", name=f"ot_h{half}"
                )
                nc.vector.tensor_copy(ot[:], ps[:, :256])
                e.dma_start(
                    out_d[384:512, 1536 + 256 * half : 1536 + 256 * (half + 1)],
                    ot[:],
                )
    nc.compile()
    return nc


def get_nc():
    if "nc" not in _cached:
        _cached["nc"] = _build_nc()
    return _cached["nc"]


def pack_inputs(x, w):
    """Host-side bit unpack + layout. Returns (xbt, wd_cores, bias).

    xbt: [128, 16, BATCH] fp8; xbt[p, kt, b] = bit kt*128+p of x[b]
    wd_cores[p]: [4, 128, 16, 512] fp8; [ob, p, kt, o'] =
        (w0-w1) at bit row kt*128+p, out feature ob*512+o'
    bias: [POP, OUT_F] int32 colsum of w1 bits
    """
    xb = np.unpackbits(
        x.view(np.uint8).reshape(BATCH, IN_INTS, 8), axis=-1, bitorder="little"
    ).reshape(BATCH, K)
    xbt = np.ascontiguousarray(
        xb.T.reshape(KT, 128, BATCH).transpose(1, 0, 2)
    ).astype(_FP8)

    wbits = np.unpackbits(
        w.view(np.uint8).reshape(POP, IN_INTS, 2, OUT_F, 8),
        axis=-1,
        bitorder="little",
    )  # [POP, IN_INTS, 2, OUT_F, 64]
    w0 = wbits[:, :, 0].transpose(0, 1, 3, 2).reshape(POP, K, OUT_F)
    w1 = wbits[:, :, 1].transpose(0, 1, 3, 2).reshape(POP, K, OUT_F)
    bias = w1.sum(axis=1, dtype=np.int32)  # [POP, OUT_F]
    wd = w0.astype(np.int8) - w1.astype(np.int8)  # {-1,0,1}
    wd_cores = [
        np.ascontiguousarray(
            wd[p].reshape(KT, 128, 4, 512).transpose(2, 1, 0, 3)
        ).astype(_FP8)
        for p in range(POP)
    ]
    return xbt, wd_cores, bias


def kernel(x, w):
    from concourse.bass_utils import run_bass_kernel_spmd

    nc = get_nc()
    xbt, wd_cores, bias = pack_inputs(np.asarray(x), np.asarray(w))
    in_maps = [{"xbt": xbt, "wd": wd_cores[p]} for p in range(N_CORES)]
    try:
        res = run_bass_kernel_spmd(nc, in_maps, list(range(N_CORES)))
    except Exception:
        # NRT_EXEC_UNIT_UNRECOVERABLE has been observed transiently on this
        # fabric; one retry has always succeeded.
        res = run_bass_kernel_spmd(nc, in_maps, list(range(N_CORES)))
    out = np.empty((POP, BATCH, OUT_F), dtype=np.int32)
    for p in range(N_CORES):
        out[p] = res.results[p]["out"].astype(np.int32) + bias[p][None, :]
    return out


# revision 11
# speedup vs baseline: 1.2002x; 1.0351x over previous
"""EvoBinarizedLayer as one fp8 matmul per population member.

Math: per population p, with xb = unpacked bits of x (LSB-first) and
w0/w1 the two unpacked weight bit-planes,

  count[p] = xb @ w0 + (1 - xb) @ w1
           = xb @ (w0 - w1) + colsum(w1)

so each core computes a single [512,2048] @ [2048,2048] matmul with
lhs entries in {0,1} and rhs entries in {-1,0,1} (both exact in fp8
e4m3, accumulated exactly in fp32 PSUM), plus a per-(p,o) bias added
on the host. Counts <= 2048 are exact in fp16, so the device emits
fp16 and the host upcasts to int32.

Sharding: population dim P=8, one member per NeuronCore (x replicated).

Device schedule (from baseline trace analysis): the warm MM stream is
already at the per-instruction roofline (216 ns per N=512 DoubleRow
matmul, LDWEIGHTS hidden), so the wins are at the edges:
  - input DMAs issued in strict need-order on the two HWDGE queues
    with per-partition-contiguous DRAM layouts (2+ KiB packets keep
    each queue at ~165 KB/us; gpsimd software-DGE is unused since its
    ring drain costs a fixed ~2.2 us in the exit barrier);
  - PE warmup matmuls start immediately after the entry barrier
    (vector memset is the first op) and bridge until data arrives,
    opening the HAM clock gate before the real stream;
  - each [128,512] PSUM drain is CAST and DMA'd out immediately (no
    [128,1024] pair staging), and the very last batch-tile of the
    last o-quarter runs as two N=256 half-groups in separate PSUM
    banks so the final dependency chain is MM -> 350ns CAST -> 64KiB
    DMA instead of MM -> 685ns CAST -> 256KiB DMA.
"""

import numpy as np
import ml_dtypes

POP, BATCH, IN_INTS, OUT_F = 8, 512, 32, 2048
K = IN_INTS * 64          # 2048 contraction (bit) dim
KT = K // 128             # 16 k-tiles of 128
N_CORES = 8
N_WARM = 30

_FP8 = ml_dtypes.float8_e4m3

_cached = {}


def _build_nc():
    import concourse.tile as tile
    from concourse import bacc, mybir

    dt = mybir.dt
    nc = bacc.Bacc(
        "TRN2", target_bir_lowering=False, debug=False, num_devices=N_CORES
    )
    xbt_d = nc.dram_tensor(
        "xbt", [128, KT, BATCH], dt.float8e4, kind="ExternalInput"
    ).ap()
    wd_d = nc.dram_tensor(
        "wd", [4, 128, KT, 512], dt.float8e4, kind="ExternalInput"
    ).ap()
    out_d = nc.dram_tensor(
        "out", [BATCH, OUT_F], dt.float16, kind="ExternalOutput"
    ).ap()

    with tile.TileContext(nc) as tc:
        with (
            tc.tile_pool(name="xbt", bufs=1) as xbt_pool,
            tc.tile_pool(name="wd", bufs=1) as wd_pool,
            tc.tile_pool(name="outp", bufs=6) as out_pool,
            tc.tile_pool(name="psum", bufs=8, space="PSUM") as psum_pool,
        ):
            S, A, G = nc.sync, nc.scalar, nc.gpsimd
            DR = mybir.MatmulPerfMode.DoubleRow

            # xbt_sb[p, k, b]: bit row k*128+p, batch b
            xbt_sb = xbt_pool.tile([128, KT, BATCH], dt.float8e4)
            # wd_sb[p, ob, k, o']: bit row k*128+p, out feature ob*512+o'
            wd_sb = wd_pool.tile([128, 4, KT, 512], dt.float8e4)

            # PE warmup: dummy DoubleRow matmuls on a small zeroed tile so
            # the HAM clock-gate opens (K=8/8) before the real stream.
            # memset on vector = its first op, so warmups start right
            # after the entry barrier and bridge until the first input
            # pieces land.
            warm = xbt_pool.tile([128, 2, 128], dt.float8e4, tag="warm")
            nc.vector.memset(warm[:], 0.0)
            wps = psum_pool.tile([128, 512], dt.float32, tag="ps")
            for _ in range(N_WARM):
                nc.tensor.matmul(
                    wps[:, :128],
                    warm[:],
                    warm[:],
                    start=True,
                    stop=True,
                    perf_mode=DR,
                )

            def dma_x(e, kt0, kt1):
                e.dma_start(xbt_sb[:, kt0:kt1, :], xbt_d[:, kt0:kt1, :])

            def dma_w(e, ob, kt0, kt1):
                e.dma_start(wd_sb[:, ob, kt0:kt1, :], wd_d[ob][:, kt0:kt1, :])

            rr = [0]
            engines = [S, A]

            def next_engine():
                e = engines[rr[0] % 2]
                rr[0] += 1
                return e

            # All DMAs ride the two HWDGE queues (sync/scalar): gpsimd's
            # software-DGE ring costs a fixed ~2.2us drain in the exit
            # barrier, and with per-partition-contiguous DRAM layouts the
            # HWDGE queues sustain ~165 KB/us each (2 KiB packets) vs the
            # ~300 KB/us stream demand. Need-order at 256 KiB k-quarter
            # granularity, x/w alternating across the two queues.
            for kq in range(4):
                dma_x(next_engine(), 4 * kq, 4 * (kq + 1))
                dma_w(next_engine(), 0, 4 * kq, 4 * (kq + 1))
            for ob in range(1, 4):
                for kq in range(4):
                    dma_w(next_engine(), ob, 4 * kq, 4 * (kq + 1))

            # Drain: CAST the [128,512] psum into a [128,1024] o-pair
            # staging tile; DMA the pair out (2 KiB runs per partition)
            # once its second half lands. Exception: (bt=3, ob=2) goes
            # out on its own right away so the final-tile staging only
            # carries the two small ob=3 halves.
            stage = {}

            def drain(ps, ob, bt, e):
                half = ob % 2
                if half == 0:
                    stage[bt] = out_pool.tile(
                        [128, 1024], dt.float16, tag="ot", name=f"ot_{ob}_{bt}"
                    )
                nc.vector.tensor_copy(
                    stage[bt][:, 512 * half : 512 * (half + 1)], ps[:]
                )
                if half == 1:
                    e.dma_start(
                        out_d[
                            128 * bt : 128 * (bt + 1),
                            1024 * (ob // 2) : 1024 * (ob // 2 + 1),
                        ],
                        stage[bt][:],
                    )

            def drain_single(ps, ob, bt, e):
                ot = out_pool.tile(
                    [128, 512], dt.float16, tag="ot", name=f"ot_{ob}_{bt}"
                )
                nc.vector.tensor_copy(ot[:], ps[:])
                e.dma_start(
                    out_d[
                        128 * bt : 128 * (bt + 1),
                        512 * ob : 512 * (ob + 1),
                    ],
                    ot[:],
                )

            # ob 0-2: k-outer across 4 concurrent psum banks (one per
            # batch tile) so each arriving chunk feeds 4 matmuls before
            # the next is needed (compute ramp matches DMA supply).
            for ob in range(3):
                pss = [
                    psum_pool.tile(
                        [128, 512], dt.float32, tag="ps", name=f"ps_{ob}_{bt}"
                    )
                    for bt in range(4)
                ]
                for k in range(KT // 2):
                    for bt in range(4):
                        nc.tensor.matmul(
                            pss[bt][:],
                            xbt_sb[:, 2 * k : 2 * k + 2, 128 * bt : 128 * (bt + 1)],
                            wd_sb[:, ob, 2 * k : 2 * k + 2, :],
                            start=(k == 0),
                            stop=(k == KT // 2 - 1),
                            perf_mode=DR,
                        )
                for bt in range(4):
                    if ob == 2:
                        # ob2 goes out immediately as singles so the
                        # final o-quarter's drains are small and the
                        # HWDGE queues are clear at the end.
                        drain_single(pss[bt], ob, bt, (A, S)[bt % 2])
                    else:
                        drain(pss[bt], ob, bt, next_engine())

            # Final o-quarter: batch-tile-serial (k inner) so drains
            # stagger; the last batch tile runs as two N=256 half-groups
            # in separate PSUM banks for a short final drain chain.
            for bt in range(3):
                ps = psum_pool.tile(
                    [128, 512], dt.float32, tag="ps", name=f"ps_l{bt}"
                )
                for k in range(KT // 2):
                    nc.tensor.matmul(
                        ps[:],
                        xbt_sb[:, 2 * k : 2 * k + 2, 128 * bt : 128 * (bt + 1)],
                        wd_sb[:, 3, 2 * k : 2 * k + 2, :],
                        start=(k == 0),
                        stop=(k == KT // 2 - 1),
                        perf_mode=DR,
                    )
                drain_single(ps, 3, bt, (A, A, S)[bt])
            for half, e in ((0, S), (1, S)):
                ps = psum_pool.tile(
                    [128, 512], dt.float32, tag="ps", name=f"ps_h{half}"
                )
                for k in range(KT // 2):
                    nc.tensor.matmul(
                        ps[:, :256],
                        xbt_sb[:, 2 * k : 2 * k + 2, 384:512],
                        wd_sb[:, 3, 2 * k : 2 * k + 2, 256 * half : 256 * (half + 1)],
                        start=(k == 0),
                        stop=(k == KT // 2 - 1),
                        perf_mode=DR,
                    )
                ot = out_pool.tile(
                    [128, 256], dt.float16, tag="ot", name=f"ot_h{half}"
                )
                nc.vector.tensor_copy(ot[:], ps[:, :256])
                e.dma_start(
                    out_d[384:512, 1536 + 256 * half : 1536 + 256 * (half + 1)],
                    ot[:],
                )
    nc.compile()
    return nc


def get_nc():
    if "nc" not in _cached:
        _cached["nc"] = _build_nc()
    return _cached["nc"]


def pack_inputs(x, w):
    """Host-side bit unpack + layout. Returns (xbt, wd_cores, bias).

    xbt: [128, 16, BATCH] fp8; xbt[p, kt, b] = bit kt*128+p of x[b]
    wd_cores[p]: [4, 128, 16, 512] fp8; [ob, p, kt, o'] =
        (w0-w1) at bit row kt*128+p, out feature ob*512+o'
    bias: [POP, OUT_F] int32 colsum of w1 bits
    """
    xb = np.unpackbits(
        x.view(np.uint8).reshape(BATCH, IN_INTS, 8), axis=-1, bitorder="little"
    ).reshape(BATCH, K)
    xbt = np.ascontiguousarray(
        xb.T.reshape(KT, 128, BATCH).transpose(1, 0, 2)
    ).astype(_FP8)

    wbits = np.unpackbits(
        w.view(np.uint8).reshape(POP, IN_INTS, 2, OUT_F, 8),
        axis=-1,
        bitorder="little",
    )  # [POP, IN_INTS, 2, OUT_F, 64]
    w0 = wbits[:, :, 0].transpose(0, 1, 3, 2).reshape(POP, K, OUT_F)
    w1 = wbits[:, :, 1].transpose(0, 1, 3, 2).reshape(POP, K, OUT_F)
    bias = w1.sum(axis=1, dtype=np.int32)  # [POP, OUT_F]
    wd = w0.astype(np.int8) - w1.astype(np.int8)  # {-1,0,1}
    wd_cores = [
        np.ascontiguousarray(
            wd[p].reshape(KT, 128, 4, 512).transpose(2, 1, 0, 3)
        ).astype(_FP8)
        for p in range(POP)
    ]
    return xbt, wd_cores, bias


def kernel(x, w):
    from concourse.bass_utils import run_bass_kernel_spmd

    nc = get_nc()
    xbt, wd_cores, bias = pack_inputs(np.asarray(x), np.asarray(w))
    in_maps = [{"xbt": xbt, "wd": wd_cores[p]} for p in range(N_CORES)]
    try:
        res = run_bass_kernel_spmd(nc, in_maps, list(range(N_CORES)))
    except Exception:
        # NRT_EXEC_UNIT_UNRECOVERABLE has been observed transiently on this
        # fabric; one retry has always succeeded.
        res = run_bass_kernel_spmd(nc, in_maps, list(range(N_CORES)))
    out = np.empty((POP, BATCH, OUT_F), dtype=np.int32)
    for p in range(N_CORES):
        out[p] = res.results[p]["out"].astype(np.int32) + bias[p][None, :]
    return out


# revision 12
# speedup vs baseline: 1.2081x; 1.0066x over previous
"""EvoBinarizedLayer as one fp8 matmul per population member.

Math: per population p, with xb = unpacked bits of x (LSB-first) and
w0/w1 the two unpacked weight bit-planes,

  count[p] = xb @ w0 + (1 - xb) @ w1
           = xb @ (w0 - w1) + colsum(w1)

so each core computes a single [512,2048] @ [2048,2048] matmul with
lhs entries in {0,1} and rhs entries in {-1,0,1} (both exact in fp8
e4m3, accumulated exactly in fp32 PSUM), plus a per-(p,o) bias added
on the host. Counts <= 2048 are exact in fp16, so the device emits
fp16 and the host upcasts to int32.

Sharding: population dim P=8, one member per NeuronCore (x replicated).

Device schedule (from baseline trace analysis): the warm MM stream is
already at the per-instruction roofline (216 ns per N=512 DoubleRow
matmul, LDWEIGHTS hidden), so the wins are at the edges:
  - input DMAs issued in strict need-order on the two HWDGE queues
    with per-partition-contiguous DRAM layouts (2+ KiB packets keep
    each queue at ~165 KB/us; gpsimd software-DGE is unused since its
    ring drain costs a fixed ~2.2 us in the exit barrier);
  - PE warmup matmuls start immediately after the entry barrier
    (vector memset is the first op) and bridge until data arrives,
    opening the HAM clock gate before the real stream;
  - each [128,512] PSUM drain is CAST and DMA'd out immediately (no
    [128,1024] pair staging), and the very last batch-tile of the
    last o-quarter runs as two N=256 half-groups in separate PSUM
    banks so the final dependency chain is MM -> 350ns CAST -> 64KiB
    DMA instead of MM -> 685ns CAST -> 256KiB DMA.
"""

import numpy as np
import ml_dtypes

POP, BATCH, IN_INTS, OUT_F = 8, 512, 32, 2048
K = IN_INTS * 64          # 2048 contraction (bit) dim
KT = K // 128             # 16 k-tiles of 128
N_CORES = 8
N_WARM = 30

_FP8 = ml_dtypes.float8_e4m3

_cached = {}


def _build_nc():
    import concourse.tile as tile
    from concourse import bacc, mybir

    dt = mybir.dt
    nc = bacc.Bacc(
        "TRN2", target_bir_lowering=False, debug=False, num_devices=N_CORES
    )
    xbt_d = nc.dram_tensor(
        "xbt", [128, KT, BATCH], dt.float8e4, kind="ExternalInput"
    ).ap()
    wd_d = nc.dram_tensor(
        "wd", [4, 128, KT, 512], dt.float8e4, kind="ExternalInput"
    ).ap()
    out_d = nc.dram_tensor(
        "out", [BATCH, OUT_F], dt.float16, kind="ExternalOutput"
    ).ap()

    with tile.TileContext(nc) as tc:
        with (
            tc.tile_pool(name="xbt", bufs=1) as xbt_pool,
            tc.tile_pool(name="wd", bufs=1) as wd_pool,
            tc.tile_pool(name="outp", bufs=6) as out_pool,
            tc.tile_pool(name="psum", bufs=8, space="PSUM") as psum_pool,
        ):
            S, A, G = nc.sync, nc.scalar, nc.gpsimd
            DR = mybir.MatmulPerfMode.DoubleRow

            # xbt_sb[p, k, b]: bit row k*128+p, batch b
            xbt_sb = xbt_pool.tile([128, KT, BATCH], dt.float8e4)
            # wd_sb[p, ob, k, o']: bit row k*128+p, out feature ob*512+o'
            wd_sb = wd_pool.tile([128, 4, KT, 512], dt.float8e4)

            # PE warmup: dummy DoubleRow matmuls on a small zeroed tile so
            # the HAM clock-gate opens (K=8/8) before the real stream.
            # memset on vector = its first op, so warmups start right
            # after the entry barrier and bridge until the first input
            # pieces land.
            warm = xbt_pool.tile([128, 2, 128], dt.float8e4, tag="warm")
            nc.vector.memset(warm[:], 0.0)
            wps = psum_pool.tile([128, 512], dt.float32, tag="ps")
            for _ in range(N_WARM):
                nc.tensor.matmul(
                    wps[:, :128],
                    warm[:],
                    warm[:],
                    start=True,
                    stop=True,
                    perf_mode=DR,
                )

            def dma_x(e, kt0, kt1):
                e.dma_start(xbt_sb[:, kt0:kt1, :], xbt_d[:, kt0:kt1, :])

            def dma_w(e, ob, kt0, kt1):
                e.dma_start(wd_sb[:, ob, kt0:kt1, :], wd_d[ob][:, kt0:kt1, :])

            rr = [0]
            engines = [S, A]

            def next_engine():
                e = engines[rr[0] % 2]
                rr[0] += 1
                return e

            # All DMAs ride the two HWDGE queues (sync/scalar): gpsimd's
            # software-DGE ring costs a fixed ~2.2us drain in the exit
            # barrier, and with per-partition-contiguous DRAM layouts the
            # HWDGE queues sustain ~165 KB/us each (2 KiB packets) vs the
            # ~300 KB/us stream demand. Need-order at 256 KiB k-quarter
            # granularity, x/w alternating across the two queues.
            # First pieces split by partition halves across both queues:
            # 64 descriptors each (~0.78 us at the ~12 ns/descriptor
            # per-queue rate) instead of 128, so the first matmul's
            # dependencies land ~0.7 us sooner.
            S.dma_start(xbt_sb[0:64, 0:4, :], xbt_d[0:64, 0:4, :])
            A.dma_start(xbt_sb[64:128, 0:4, :], xbt_d[64:128, 0:4, :])
            S.dma_start(wd_sb[0:64, 0, 0:4, :], wd_d[0][0:64, 0:4, :])
            A.dma_start(wd_sb[64:128, 0, 0:4, :], wd_d[0][64:128, 0:4, :])
            for kq in range(1, 4):
                dma_x(next_engine(), 4 * kq, 4 * (kq + 1))
                dma_w(next_engine(), 0, 4 * kq, 4 * (kq + 1))
            for ob in range(1, 4):
                for kq in range(4):
                    dma_w(next_engine(), ob, 4 * kq, 4 * (kq + 1))

            # Drain: CAST the [128,512] psum into a [128,1024] o-pair
            # staging tile; DMA the pair out (2 KiB runs per partition)
            # once its second half lands. Exception: (bt=3, ob=2) goes
            # out on its own right away so the final-tile staging only
            # carries the two small ob=3 halves.
            stage = {}

            def drain(ps, ob, bt, e):
                half = ob % 2
                if half == 0:
                    stage[bt] = out_pool.tile(
                        [128, 1024], dt.float16, tag="ot", name=f"ot_{ob}_{bt}"
                    )
                nc.vector.tensor_copy(
                    stage[bt][:, 512 * half : 512 * (half + 1)], ps[:]
                )
                if half == 1:
                    e.dma_start(
                        out_d[
                            128 * bt : 128 * (bt + 1),
                            1024 * (ob // 2) : 1024 * (ob // 2 + 1),
                        ],
                        stage[bt][:],
                    )

            def drain_single(ps, ob, bt, e):
                ot = out_pool.tile(
                    [128, 512], dt.float16, tag="ot", name=f"ot_{ob}_{bt}"
                )
                nc.vector.tensor_copy(ot[:], ps[:])
                e.dma_start(
                    out_d[
                        128 * bt : 128 * (bt + 1),
                        512 * ob : 512 * (ob + 1),
                    ],
                    ot[:],
                )

            # ob 0-2: k-outer across 4 concurrent psum banks (one per
            # batch tile) so each arriving chunk feeds 4 matmuls before
            # the next is needed (compute ramp matches DMA supply).
            for ob in range(3):
                pss = [
                    psum_pool.tile(
                        [128, 512], dt.float32, tag="ps", name=f"ps_{ob}_{bt}"
                    )
                    for bt in range(4)
                ]
                for k in range(KT // 2):
                    for bt in range(4):
                        nc.tensor.matmul(
                            pss[bt][:],
                            xbt_sb[:, 2 * k : 2 * k + 2, 128 * bt : 128 * (bt + 1)],
                            wd_sb[:, ob, 2 * k : 2 * k + 2, :],
                            start=(k == 0),
                            stop=(k == KT // 2 - 1),
                            perf_mode=DR,
                        )
                for bt in range(4):
                    if ob == 2:
                        # ob2 goes out immediately as singles so the
                        # final o-quarter's drains are small and the
                        # HWDGE queues are clear at the end.
                        drain_single(pss[bt], ob, bt, (A, S)[bt % 2])
                    else:
                        drain(pss[bt], ob, bt, next_engine())

            # Final o-quarter: batch-tile-serial (k inner) so drains
            # stagger; the last batch tile runs as two N=256 half-groups
            # in separate PSUM banks for a short final drain chain.
            for bt in range(3):
                ps = psum_pool.tile(
                    [128, 512], dt.float32, tag="ps", name=f"ps_l{bt}"
                )
                for k in range(KT // 2):
                    nc.tensor.matmul(
                        ps[:],
                        xbt_sb[:, 2 * k : 2 * k + 2, 128 * bt : 128 * (bt + 1)],
                        wd_sb[:, 3, 2 * k : 2 * k + 2, :],
                        start=(k == 0),
                        stop=(k == KT // 2 - 1),
                        perf_mode=DR,
                    )
                drain_single(ps, 3, bt, (A, A, S)[bt])
            for half, e in ((0, S), (1, S)):
                ps = psum_pool.tile(
                    [128, 512], dt.float32, tag="ps", name=f"ps_h{half}"
                )
                for k in range(KT // 2):
                    nc.tensor.matmul(
                        ps[:, :256],
                        xbt_sb[:, 2 * k : 2 * k + 2, 384:512],
                        wd_sb[:, 3, 2 * k : 2 * k + 2, 256 * half : 256 * (half + 1)],
                        start=(k == 0),
                        stop=(k == KT // 2 - 1),
                        perf_mode=DR,
                    )
                ot = out_pool.tile(
                    [128, 256], dt.float16, tag="ot", name=f"ot_h{half}"
                )
                nc.vector.tensor_copy(ot[:], ps[:, :256])
                e.dma_start(
                    out_d[384:512, 1536 + 256 * half : 1536 + 256 * (half + 1)],
                    ot[:],
                )
    nc.compile()
    return nc


def get_nc():
    if "nc" not in _cached:
        _cached["nc"] = _build_nc()
    return _cached["nc"]


def pack_inputs(x, w):
    """Host-side bit unpack + layout. Returns (xbt, wd_cores, bias).

    xbt: [128, 16, BATCH] fp8; xbt[p, kt, b] = bit kt*128+p of x[b]
    wd_cores[p]: [4, 128, 16, 512] fp8; [ob, p, kt, o'] =
        (w0-w1) at bit row kt*128+p, out feature ob*512+o'
    bias: [POP, OUT_F] int32 colsum of w1 bits
    """
    xb = np.unpackbits(
        x.view(np.uint8).reshape(BATCH, IN_INTS, 8), axis=-1, bitorder="little"
    ).reshape(BATCH, K)
    xbt = np.ascontiguousarray(
        xb.T.reshape(KT, 128, BATCH).transpose(1, 0, 2)
    ).astype(_FP8)

    wbits = np.unpackbits(
        w.view(np.uint8).reshape(POP, IN_INTS, 2, OUT_F, 8),
        axis=-1,
        bitorder="little",
    )  # [POP, IN_INTS, 2, OUT_F, 64]
    w0 = wbits[:, :, 0].transpose(0, 1, 3, 2).reshape(POP, K, OUT_F)
    w1 = wbits[:, :, 1].transpose(0, 1, 3, 2).reshape(POP, K, OUT_F)
    bias = w1.sum(axis=1, dtype=np.int32)  # [POP, OUT_F]
    wd = w0.astype(np.int8) - w1.astype(np.int8)  # {-1,0,1}
    wd_cores = [
        np.ascontiguousarray(
            wd[p].reshape(KT, 128, 4, 512).transpose(2, 1, 0, 3)
        ).astype(_FP8)
        for p in range(POP)
    ]
    return xbt, wd_cores, bias


def kernel(x, w):
    from concourse.bass_utils import run_bass_kernel_spmd

    nc = get_nc()
    xbt, wd_cores, bias = pack_inputs(np.asarray(x), np.asarray(w))
    in_maps = [{"xbt": xbt, "wd": wd_cores[p]} for p in range(N_CORES)]
    try:
        res = run_bass_kernel_spmd(nc, in_maps, list(range(N_CORES)))
    except Exception:
        # NRT_EXEC_UNIT_UNRECOVERABLE has been observed transiently on this
        # fabric; one retry has always succeeded.
        res = run_bass_kernel_spmd(nc, in_maps, list(range(N_CORES)))
    out = np.empty((POP, BATCH, OUT_F), dtype=np.int32)
    for p in range(N_CORES):
        out[p] = res.results[p]["out"].astype(np.int32) + bias[p][None, :]
    return out
